# revision 2
# baseline (speedup 1.0000x reference)
"""nn_LphaLoss kernel — full pipeline on 8 TRN2 NeuronCores via Bass/Tile.

Per core (2 of 16 images = 128 blocks/stream): VGG conv3_1 features for
pred1/target 32x32 blocks (bf16 matmuls, contiguous flat-window rhs,
K-packed taps), 8x8 2D-DFT via [80,64] matmuls (junk pitch rows zeroed in
the DFT matrix), FFT-phase via atan2 on device, cosine similarity -> block
mask, masked L1 partial sums. Host sums 8 scalar pairs and divides.

kernel(**inputs) takes FULL inputs, returns the FULL (scalar) output.
"""
import numpy as np

BS = 32
N_CORES = 8
NB = 8            # blocks per stream per iteration
NITER = 16        # 128 blocks per stream / NB
MEAN = np.array([0.485, 0.456, 0.406], dtype=np.float32).reshape(1, 3, 1, 1)
STD = np.array([0.229, 0.224, 0.225], dtype=np.float32).reshape(1, 3, 1, 1)

_COMPILED = {}
LAST_EXEC_NS = None
PROFILE = False          # when True, run with NTFF tracing; LAST_EXEC_NS = device time
LAST_TRACE_PATH = None


def _build_nc():
    import concourse.mybir as mybir
    from concourse import bacc
    from concourse.tile import TileContext

    F32 = mybir.dt.float32
    BF16 = mybir.dt.bfloat16
    ALU = mybir.AluOpType
    ACTF = mybir.ActivationFunctionType
    AXL = mybir.AxisListType
    PI = float(np.pi)

    nc = bacc.Bacc("TRN2", target_bir_lowering=False)
    dp = nc.declare_dram_parameter
    xa_d = dp("xa", [3, 128, 1296], BF16, isOutput=False)   # pred1 blocks, 36x36 padded
    xb_d = dp("xb", [3, 128, 1296], BF16, isOutput=False)   # target blocks
    pq_d = dp("pq", [96, 4096], BF16, isOutput=False)       # pred2 blocked [3c*32r, 128b*32col]
    tq_d = dp("tq", [96, 4096], BF16, isOutput=False)       # target blocked
    w27_d = dp("w27", [27, 64], BF16, isOutput=False)
    w12p_d = dp("w12p", [3, 128, 64], BF16, isOutput=False)
    w12s_d = dp("w12s", [3, 64, 64], BF16, isOutput=False)
    w21p_d = dp("w21p", [3, 128, 128], BF16, isOutput=False)
    w21s_d = dp("w21s", [3, 64, 128], BF16, isOutput=False)
    w22_d = dp("w22", [9, 128, 128], BF16, isOutput=False)
    w5_d = dp("w5", [9, 128, 256], BF16, isOutput=False)
    dftr_d = dp("dftr", [80, 64], BF16, isOutput=False)
    dfti_d = dp("dfti", [80, 64], BF16, isOutput=False)
    b1_d = dp("b1", [64, 1], F32, isOutput=False)
    b2_d = dp("b2", [64, 1], F32, isOutput=False)
    b3_d = dp("b3", [128, 1], F32, isOutput=False)
    b4_d = dp("b4", [128, 1], F32, isOutput=False)
    b5bc_d = dp("b5bc", [80, 256], F32, isOutput=False)
    o_d = dp("o", [1, 2], F32, isOutput=True)

    with TileContext(nc) as tc:
        with (
            tc.tile_pool(name="const", bufs=1) as cp,
            tc.tile_pool(name="xin", bufs=2) as xp,
            tc.tile_pool(name="mid", bufs=1) as mp,
            tc.tile_pool(name="hph", bufs=2) as hp,
            tc.tile_pool(name="tmp", bufs=1) as tp,
            tc.tile_pool(name="acc", bufs=1) as ac,
            tc.tile_pool(name="psa", bufs=3, space="PSUM") as pap,   # [64,504] 1 bank
            tc.tile_pool(name="psb", bufs=3, space="PSUM") as pbp,   # [128,288] 1 bank
            tc.tile_pool(name="psd", bufs=1, space="PSUM") as pdp,   # pr/pi 1 bank each
        ):
            # ---- constants ----
            w27t = cp.tile([27, 64], BF16)
            nc.sync.dma_start(w27t[:, :], w27_d[:, :])
            w12pt = cp.tile([128, 3, 64], BF16)
            nc.sync.dma_start(w12pt[:, :, :], w12p_d[:, :, :].rearrange("a p m -> p a m"))
            w12st = cp.tile([64, 3, 64], BF16)
            nc.sync.dma_start(w12st[:, :, :], w12s_d[:, :, :].rearrange("a p m -> p a m"))
            w21pt = cp.tile([128, 3, 128], BF16)
            nc.sync.dma_start(w21pt[:, :, :], w21p_d[:, :, :].rearrange("a p m -> p a m"))
            w21st = cp.tile([64, 3, 128], BF16)
            nc.sync.dma_start(w21st[:, :, :], w21s_d[:, :, :].rearrange("a p m -> p a m"))
            w22t = cp.tile([128, 9, 128], BF16)
            nc.sync.dma_start(w22t[:, :, :], w22_d[:, :, :].rearrange("a p m -> p a m"))
            w5t = cp.tile([128, 9, 256], BF16)
            nc.sync.dma_start(w5t[:, :, :], w5_d[:, :, :].rearrange("a p m -> p a m"))
            dftrt = cp.tile([80, 64], BF16)
            nc.sync.dma_start(dftrt[:, :], dftr_d[:, :])
            dftit = cp.tile([80, 64], BF16)
            nc.sync.dma_start(dftit[:, :], dfti_d[:, :])
            b1t = cp.tile([64, 1], F32)
            nc.sync.dma_start(b1t[:, :], b1_d[:, :])
            b2t = cp.tile([64, 1], F32)
            nc.sync.dma_start(b2t[:, :], b2_d[:, :])
            b3t = cp.tile([128, 1], F32)
            nc.sync.dma_start(b3t[:, :], b3_d[:, :])
            b4t = cp.tile([128, 1], F32)
            nc.sync.dma_start(b4t[:, :], b4_d[:, :])
            b5bct = cp.tile([80, 256], F32)
            nc.sync.dma_start(b5bct[:, :], b5bc_d[:, :])
            ones64t = cp.tile([64, 1], F32)
            nc.gpsimd.memset(ones64t[:, :], 1.0)
            ones96t = cp.tile([96, 1], F32)
            nc.gpsimd.memset(ones96t[:, :], 1.0)

            # ---- accumulators ----
            STATS = ac.tile([1, NITER * 24], F32)   # [0, j*24 + stat*8 + t]
            L1S = ac.tile([96, 128], F32)           # per-block |p2-tg| partial sums

            # ---- persistent padded mid tiles (borders zeroed once) ----
            Y1PD = ac.tile([128, NB, 1160], BF16)
            Y2PD = ac.tile([128, NB, 328], BF16)
            Y3P = ac.tile([128, NB, 328], BF16)
            Y4P = ac.tile([128, NB, 104], BF16)
            nc.gpsimd.memset(Y1PD[:, :, 0:34], 0.0)
            nc.gpsimd.memset(Y1PD[:, :, 1122:1160], 0.0)
            nc.gpsimd.memset(
                Y1PD[:, :, 0:1156].rearrange(
                    "p b (r c) -> p b r c", c=34)[:, :, :, 0:34:33], 0.0)
            nc.gpsimd.memset(Y2PD[:, :, 0:18], 0.0)
            nc.gpsimd.memset(Y2PD[:, :, 306:328], 0.0)
            nc.gpsimd.memset(
                Y2PD[:, :, 0:324].rearrange(
                    "p b (r c) -> p b r c", c=18)[:, :, :, 0:18:17], 0.0)
            nc.gpsimd.memset(Y3P[:, :, 0:18], 0.0)
            nc.gpsimd.memset(Y3P[:, :, 306:328], 0.0)
            nc.gpsimd.memset(
                Y3P[:, :, 0:324].rearrange(
                    "p b (r c) -> p b r c", c=18)[:, :, :, 0:18:17], 0.0)
            nc.gpsimd.memset(Y4P[:, :, 0:10], 0.0)
            nc.gpsimd.memset(Y4P[:, :, 90:104], 0.0)
            nc.gpsimd.memset(
                Y4P[:, :, 0:100].rearrange(
                    "p b (r c) -> p b r c", c=10)[:, :, :, 0:10:9], 0.0)

            # ---- main loop ----
            for j in range(NITER):
                b0 = j * NB
                Ps = []
                for s, src_d in ((0, xa_d), (1, xb_d)):
                    # im2col: 9 shifted DMAs -> X27 [27, NB, 1224] (36-pitch planes)
                    X27 = xp.tile([27, NB, 1224], BF16, tag="x27")
                    for k in range(9):
                        dy, dx = k // 3, k % 3
                        off = dy * 36 + dx
                        nc.sync.dma_start(
                            X27[3 * k:3 * k + 3, :, 0:1222],
                            src_d[:, b0:b0 + NB, off:off + 1222])

                    # conv1_1 -> Y1PD (persistent, borders pre-zeroed)
                    for t in range(NB):
                        for r0, nr in ((0, 14), (14, 14), (28, 4)):  # row chunks
                            pA = pap.tile([64, 504], F32, tag="pa")
                            nc.tensor.matmul(
                                pA[:, 0:nr * 36], w27t[:, :],
                                X27[:, t, r0 * 36:(r0 + nr) * 36],
                                start=True, stop=True)
                            dst = Y1PD[0:64, t, 35 + r0 * 34:35 + (r0 + nr) * 34].rearrange(
                                "p (r c) -> p r c", c=34)[:, :, 0:32]
                            nc.scalar.activation(
                                dst,
                                pA[:, 0:nr * 36].rearrange(
                                    "p (r c) -> p r c", c=36)[:, :, 0:32],
                                ACTF.Relu, bias=b1t[:, 0:1])
                    nc.sync.dma_start(
                        Y1PD[64:128, :, 0:1155], Y1PD[0:64, :, 1:1156])

                    # conv1_2 (+bias2+relu+pool1) -> Y2PD (persistent)
                    for t in range(NB):
                        for r0, nr in ((0, 14), (14, 14), (28, 4)):  # row chunks
                            pB = pap.tile([64, 504], F32, tag="pa")
                            nw = nr * 34
                            base = r0 * 34
                            for dy in range(3):
                                nc.tensor.matmul(
                                    pB[:, 0:nw], w12pt[:, dy, :],
                                    Y1PD[:, t, base + dy * 34:base + dy * 34 + nw],
                                    start=(dy == 0), stop=False)
                            for dy in range(3):
                                nc.tensor.matmul(
                                    pB[:, 0:nw], w12st[:, dy, :],
                                    Y1PD[0:64, t, base + dy * 34 + 2:base + dy * 34 + 2 + nw],
                                    start=False, stop=(dy == 2))
                            pool_in = pB[:, 0:nw].rearrange(
                                "p (u v) -> p u v", v=34)[:, :, 0:32].rearrange(
                                "p (r dr) (c dc) -> p r c dr dc", dr=2, dc=2)
                            t3 = tp.tile([64, 7, 16], F32, tag="pl")
                            nc.vector.tensor_reduce(t3[:, 0:nr // 2, :], pool_in,
                                                    axis=AXL.XY, op=ALU.max)
                            dst = Y2PD[0:64, t, 19 + (r0 // 2) * 18:19 + (r0 // 2 + nr // 2) * 18].rearrange(
                                "p (r c) -> p r c", c=18)[:, :, 0:16]
                            nc.vector.tensor_scalar(
                                out=dst, in0=t3[:, 0:nr // 2, :], scalar1=b2t[:, 0:1],
                                scalar2=0.0, op0=ALU.add, op1=ALU.max)
                    nc.sync.dma_start(
                        Y2PD[64:128, :, 0:323], Y2PD[0:64, :, 1:324])

                    # conv2_1 -> Y3P (persistent)
                    for t in range(NB):
                        pC = pbp.tile([128, 288], F32, tag="pb")
                        for dy in range(3):
                            nc.tensor.matmul(
                                pC[:, :], w21pt[:, dy, :],
                                Y2PD[:, t, dy * 18:dy * 18 + 288],
                                start=(dy == 0), stop=False)
                        for dy in range(3):
                            nc.tensor.matmul(
                                pC[:, :], w21st[:, dy, :],
                                Y2PD[0:64, t, dy * 18 + 2:dy * 18 + 2 + 288],
                                start=False, stop=(dy == 2))
                        dst = Y3P[:, t, 19:19 + 16 * 18].rearrange(
                            "p (r c) -> p r c", c=18)[:, :, 0:16]
                        nc.scalar.activation(
                            dst,
                            pC[:, :].rearrange("p (r c) -> p r c", c=18)[:, :, 0:16],
                            ACTF.Relu, bias=b3t[:, 0:1])

                    # conv2_2 (+bias4+relu+pool2) -> Y4P (persistent)
                    for t in range(NB):
                        pD = pbp.tile([128, 288], F32, tag="pb")
                        for k in range(9):
                            dy, dx = k // 3, k % 3
                            nc.tensor.matmul(
                                pD[:, :], w22t[:, k, :],
                                Y3P[:, t, dy * 18 + dx:dy * 18 + dx + 288],
                                start=(k == 0), stop=(k == 8))
                        pool_in = pD[:, :].rearrange(
                            "p (u v) -> p u v", v=18)[:, :, 0:16].rearrange(
                            "p (r dr) (c dc) -> p r c dr dc", dr=2, dc=2)
                        t3 = tp.tile([128, 8, 8], F32, tag="ql")
                        nc.vector.tensor_reduce(t3[:, :, :], pool_in,
                                                axis=AXL.XY, op=ALU.max)
                        dst = Y4P[:, t, 11:11 + 8 * 10].rearrange(
                            "p (r c) -> p r c", c=10)[:, :, 0:8]
                        nc.vector.tensor_scalar(
                            out=dst, in0=t3[:, :, :], scalar1=b4t[:, 0:1],
                            scalar2=0.0, op0=ALU.add, op1=ALU.max)

                    # conv3_1 transposed (data stationary): out [80sp(10-pitch), 256ch]
                    # + b5 broadcast -> H80; junk pitch rows killed later by dft rows=0
                    H80 = hp.tile([80, NB, 256], BF16, tag="h")
                    for t in range(NB):
                        pH = pbp.tile([80, 256], F32, tag="pb")
                        for k in range(9):
                            dy, dx = k // 3, k % 3
                            nc.tensor.matmul(
                                pH[:, :], Y4P[:, t, dy * 10 + dx:dy * 10 + dx + 80],
                                w5t[:, k, :], start=(k == 0), stop=(k == 8))
                        nc.vector.tensor_tensor(out=H80[:, t, :], in0=pH[:, :],
                                                in1=b5bct[:, :], op=ALU.add)

                    # DFT (8x8, real+imag), batched: chunks of 512 over flat blocks*ch
                    PRb = tp.tile([64, NB, 256], F32, tag="prb")
                    PIb = tp.tile([64, NB, 256], F32, tag="pib")
                    H80f = H80[:, :, :].rearrange("p a b -> p (a b)")
                    PRbf = PRb[:, :, :].rearrange("p a b -> p (a b)")
                    PIbf = PIb[:, :, :].rearrange("p a b -> p (a b)")
                    for c0 in range(0, NB * 256, 512):
                        pR = pdp.tile([64, 512], F32, tag="pr")
                        nc.tensor.matmul(pR[:, :], dftrt[:, :], H80f[:, c0:c0 + 512],
                                         start=True, stop=True)
                        pI = pdp.tile([64, 512], F32, tag="pi")
                        nc.tensor.matmul(pI[:, :], dftit[:, :], H80f[:, c0:c0 + 512],
                                         start=True, stop=True)
                        nc.scalar.activation(PRbf[:, c0:c0 + 512], pR[:, :], ACTF.Copy)
                        nc.vector.tensor_copy(PIbf[:, c0:c0 + 512], pI[:, :])

                    PRf = PRb[:, :, :].rearrange("p a b -> p (a b)")
                    PIf = PIb[:, :, :].rearrange("p a b -> p (a b)")
                    rinv = tp.tile([64, NB * 256], F32, tag="phD")
                    nc.vector.reciprocal_approx_fast(rinv[:, :], PRf)
                    tq_t = tp.tile([64, NB * 256], F32, tag="phA")
                    nc.vector.tensor_tensor(out=tq_t[:, :], in0=PIf, in1=rinv[:, :], op=ALU.mult)
                    ta = tp.tile([64, NB * 256], F32, tag="phB")
                    nc.scalar.activation(ta[:, :], tq_t[:, :], ACTF.Arctan)
                    tneg = tp.tile([64, NB * 256], F32, tag="phC")
                    nc.vector.tensor_scalar(out=tneg[:, :], in0=PRf,
                                            scalar1=0.0, scalar2=None, op0=ALU.is_lt)
                    tt_ = tp.tile([64, NB * 256], F32, tag="phA")
                    nc.vector.scalar_tensor_tensor(
                        out=tt_[:, :], in0=PIf, scalar=0.0, in1=tneg[:, :],
                        op0=ALU.is_ge, op1=ALU.mult)
                    tu = tp.tile([64, NB * 256], F32, tag="phD")
                    nc.vector.tensor_tensor(out=tu[:, :], in0=tneg[:, :],
                                            in1=tt_[:, :], op=ALU.subtract)
                    tv = tp.tile([64, NB * 256], F32, tag="phC")
                    nc.vector.tensor_tensor(out=tv[:, :], in0=tt_[:, :],
                                            in1=tu[:, :], op=ALU.subtract)
                    P = hp.tile([64, NB * 256], F32, tag=f"p{s}")
                    nc.vector.scalar_tensor_tensor(
                        out=P[:, :], in0=tv[:, :], scalar=PI, in1=ta[:, :],
                        op0=ALU.mult, op1=ALU.add)
                    Ps.append(P)

                # cosine-sim stats: per-block sums of p1*p2, p1^2, p2^2
                R3 = tp.tile([64, 3, NB], F32, tag="r3")
                pm = tp.tile([64, NB * 256], F32, tag="phD")
                for si, (ia, ib) in enumerate(((0, 1), (0, 0), (1, 1))):
                    nc.vector.tensor_tensor(out=pm[:, :], in0=Ps[ia][:, :],
                                            in1=Ps[ib][:, :], op=ALU.mult)
                    nc.vector.tensor_reduce(
                        R3[:, si, :], pm[:, :].rearrange("p (a b) -> p a b", a=NB),
                        axis=AXL.X, op=ALU.add)
                pS = pdp.tile([1, 3 * NB], F32, tag="pr")
                nc.tensor.matmul(pS[:, :], ones64t[:, :],
                                 R3[:, :, :].rearrange("p a b -> p (a b)"),
                                 start=True, stop=True)
                nc.vector.tensor_copy(STATS[:, j * 24:(j + 1) * 24], pS[:, :])

                # L1 partial sums for this iteration's 8 blocks
                pqc = xp.tile([96, 256], BF16, tag="pqc")
                nc.sync.dma_start(pqc[:, :], pq_d[:, j * 256:(j + 1) * 256])
                tqc = xp.tile([96, 256], BF16, tag="tqc")
                nc.sync.dma_start(tqc[:, :], tq_d[:, j * 256:(j + 1) * 256])
                dl = tp.tile([96, 256], F32, tag="dl")
                nc.vector.tensor_tensor(out=dl[:, :], in0=pqc[:, :],
                                        in1=tqc[:, :], op=ALU.subtract)
                nc.vector.tensor_reduce(
                    L1S[:, j * 8:(j + 1) * 8],
                    dl[:, :].rearrange("p (a b) -> p a b", a=8),
                    axis=AXL.X, op=ALU.add, apply_absolute_value=True)

            # ---- epilogue (all tiny, partition 0) ----
            pL = pdp.tile([1, 128], F32, tag="pr")
            nc.tensor.matmul(pL[:, :], ones96t[:, :], L1S[:, :], start=True, stop=True)
            R1 = tp.tile([1, 128], F32, tag="r1")
            nc.vector.tensor_copy(R1[:, :], pL[:, :])
            SV = STATS[:, :].rearrange("p (j s t) -> p j s t", s=3, t=8)
            NUM = SV[:, :, 0, :]
            N1 = SV[:, :, 1, :]
            N2 = SV[:, :, 2, :]
            W_ = tp.tile([1, 16, 8], F32, tag="w_")
            nc.vector.tensor_tensor(out=W_[:, :, :], in0=N1, in1=N2, op=ALU.mult)
            S2 = tp.tile([1, 16, 8], F32, tag="s2")
            nc.vector.tensor_tensor(out=S2[:, :, :], in0=NUM, in1=NUM, op=ALU.mult)
            C2 = tp.tile([1, 16, 8], F32, tag="c2")
            nc.vector.scalar_tensor_tensor(
                out=C2[:, :, :], in0=S2[:, :, :], scalar=25.0, in1=W_[:, :, :],
                op0=ALU.mult, op1=ALU.is_ge)
            M1 = tp.tile([1, 16, 8], F32, tag="m1")
            nc.vector.tensor_scalar(out=M1[:, :, :], in0=NUM, scalar1=0.0,
                                    scalar2=None, op0=ALU.is_gt)
            MASK = tp.tile([1, 128], F32, tag="mask")
            nc.vector.tensor_tensor(
                out=MASK[:, :].rearrange("p (a b) -> p a b", a=16),
                in0=C2[:, :, :], in1=M1[:, :, :], op=ALU.mult)
            PR_ = tp.tile([1, 128], F32, tag="pr_")
            nc.vector.tensor_tensor(out=PR_[:, :], in0=MASK[:, :],
                                    in1=R1[:, :], op=ALU.mult)
            OT = tp.tile([1, 2], F32, tag="ot")
            nc.vector.tensor_reduce(OT[:, 0:1], PR_[:, :], axis=AXL.X, op=ALU.add)
            nc.vector.tensor_reduce(OT[:, 1:2], MASK[:, :], axis=AXL.X, op=ALU.add)
            nc.sync.dma_start(o_d[:, :], OT[:, :])
    nc.compile()
    return nc


def _blocks(x, c):
    # [16, c, 256, 256] -> [1024, c, 32, 32], block = img*64 + by*8 + bx
    return (x.reshape(16, c, 8, 32, 8, 32).transpose(0, 2, 4, 1, 3, 5)
            .reshape(1024, c, 32, 32))


def kernel(pred1, pred2, target, w1, b1, w2, b2, w3, b3, w4, b4, w5, b5):
    import ml_dtypes
    from concourse.bass_utils import run_bass_kernel_spmd
    bf = ml_dtypes.bfloat16

    pred1 = np.asarray(pred1, np.float32)
    pred2 = np.asarray(pred2, np.float32)
    target = np.asarray(target, np.float32)
    w1, w2, w3, w4, w5 = (np.asarray(w, np.float32) for w in (w1, w2, w3, w4, w5))
    b1, b2, b3, b4, b5 = (np.asarray(b, np.float32) for b in (b1, b2, b3, b4, b5))

    def taps_T(w):  # [O, I, 3, 3] -> [9, I, O]
        return np.ascontiguousarray(
            w.transpose(2, 3, 1, 0).reshape(9, w.shape[1], w.shape[0]))

    w1T, w2T, w3T, w4T, w5T = map(taps_T, (w1, w2, w3, w4, w5))
    w27 = np.ascontiguousarray(w1T.reshape(27, 64)).astype(bf)
    w12p = np.stack([np.concatenate([w2T[3 * dy], w2T[3 * dy + 1]], 0)
                     for dy in range(3)]).astype(bf)
    w12s = np.stack([w2T[3 * dy + 2] for dy in range(3)]).astype(bf)
    w21p = np.stack([np.concatenate([w3T[3 * dy], w3T[3 * dy + 1]], 0)
                     for dy in range(3)]).astype(bf)
    w21s = np.stack([w3T[3 * dy + 2] for dy in range(3)]).astype(bf)
    w22 = w4T.astype(bf)
    w5p = w5T.astype(bf)

    idx = np.arange(8)
    s_in = (idx[:, None, None, None] * idx[None, None, :, None]
            + idx[None, :, None, None] * idx[None, None, None, :])  # [ri,ci,ro,co]
    theta = 2.0 * np.pi * (s_in % 8) / 8.0
    M_r = np.cos(theta).reshape(64, 64).astype(np.float32)
    M_i = (-np.sin(theta)).reshape(64, 64).astype(np.float32)
    M_r[np.abs(M_r) < 1e-6] = 0.0
    M_i[np.abs(M_i) < 1e-6] = 0.0
    # 10-pitch layout: row p = r*10 + c holds spatial (r,c); pitch cols 8,9 = 0
    dftr = np.zeros((80, 64), np.float32)
    dfti = np.zeros((80, 64), np.float32)
    for r in range(8):
        dftr[r * 10:r * 10 + 8, :] = M_r[r * 8:(r + 1) * 8, :]
        dfti[r * 10:r * 10 + 8, :] = M_i[r * 8:(r + 1) * 8, :]
    b5bc = np.tile(b5.reshape(1, 256), (80, 1)).astype(np.float32)

    shared = dict(
        w27=w27, w12p=w12p, w12s=w12s, w21p=w21p, w21s=w21s, w22=w22, w5=w5p,
        dftr=dftr.astype(bf), dfti=dfti.astype(bf),
        b1=np.ascontiguousarray(b1.reshape(64, 1)),
        b2=np.ascontiguousarray(b2.reshape(64, 1)),
        b3=np.ascontiguousarray(b3.reshape(128, 1)),
        b4=np.ascontiguousarray(b4.reshape(128, 1)), b5bc=b5bc)

    p1n = ((pred1 - MEAN) / STD).astype(np.float32)
    tgn = ((target - MEAN) / STD).astype(np.float32)
    bl1 = _blocks(p1n, 3)
    blt = _blocks(tgn, 3)
    blp2 = _blocks(pred2, 3)
    bltg = _blocks(target, 3)

    def pad36(blk):  # [128, 3, 32, 32] -> [3, 128, 1296] bf16
        out = np.zeros((3, 128, 36, 36), np.float32)
        out[:, :, 2:34, 2:34] = blk.transpose(1, 0, 2, 3)
        return out.reshape(3, 128, 1296).astype(bf)

    def l1fmt(blk):  # [128, 3, 32, 32] -> [96, 4096] bf16
        return np.ascontiguousarray(
            blk.transpose(1, 2, 0, 3).reshape(96, 128 * 32)).astype(bf)

    in_maps = []
    for c in range(N_CORES):
        s = slice(c * 128, (c + 1) * 128)
        in_maps.append(dict(
            xa=pad36(bl1[s]), xb=pad36(blt[s]),
            pq=l1fmt(blp2[s]), tq=l1fmt(bltg[s]), **shared))

    if "nc" not in _COMPILED:
        _COMPILED["nc"] = _build_nc()
    nc = _COMPILED["nc"]

    import time as _time
    t0 = _time.perf_counter()
    try:
        res = run_bass_kernel_spmd(nc, in_maps, list(range(N_CORES)), trace=PROFILE)
    except ModuleNotFoundError:
        res = run_bass_kernel_spmd(nc, in_maps, list(range(N_CORES)))
    global LAST_EXEC_NS, LAST_TRACE_PATH
    LAST_EXEC_NS = int((_time.perf_counter() - t0) * 1e9)
    if res.exec_time_ns:
        LAST_EXEC_NS = int(res.exec_time_ns)
    if res.instructions_and_trace:
        LAST_TRACE_PATH = res.instructions_and_trace[1]

    l1 = 0.0
    cnt = 0.0
    for c in range(N_CORES):
        o = res.results[c]["o"]
        l1 += float(o[0, 0])
        cnt += float(o[0, 1])
    out = np.float32(l1 / (cnt * 1024.0 + 1e-6))
    return np.array(out, dtype=np.float32)


# revision 3
# speedup vs baseline: 1.2512x; 1.2512x over previous
"""nn_LphaLoss kernel — full pipeline on 8 TRN2 NeuronCores via Bass/Tile.

Per core (2 of 16 images = 128 blocks/stream): VGG conv3_1 features for
pred1/target 32x32 blocks (bf16 matmuls, contiguous flat-window rhs,
K-packed taps), 8x8 2D-DFT via [80,64] matmuls (junk pitch rows zeroed in
the DFT matrix), FFT-phase via atan2 on device, cosine similarity -> block
mask, masked L1 partial sums. Host sums 8 scalar pairs and divides.

kernel(**inputs) takes FULL inputs, returns the FULL (scalar) output.
"""
import numpy as np

BS = 32
N_CORES = 8
NB = 8            # blocks per stream per iteration
NITER = 16        # 128 blocks per stream / NB
MEAN = np.array([0.485, 0.456, 0.406], dtype=np.float32).reshape(1, 3, 1, 1)
STD = np.array([0.229, 0.224, 0.225], dtype=np.float32).reshape(1, 3, 1, 1)

_COMPILED = {}
LAST_EXEC_NS = None
PROFILE = False          # when True, run with NTFF tracing; LAST_EXEC_NS = device time
LAST_TRACE_PATH = None


def _build_nc():
    import concourse.mybir as mybir
    from concourse import bacc
    from concourse.tile import TileContext

    F32 = mybir.dt.float32
    BF16 = mybir.dt.bfloat16
    ALU = mybir.AluOpType
    ACTF = mybir.ActivationFunctionType
    AXL = mybir.AxisListType
    PI = float(np.pi)

    nc = bacc.Bacc("TRN2", target_bir_lowering=False)
    dp = nc.declare_dram_parameter
    xa_d = dp("xa", [3, 128, 1296], BF16, isOutput=False)   # pred1 blocks, 36x36 padded
    xb_d = dp("xb", [3, 128, 1296], BF16, isOutput=False)   # target blocks
    pq_d = dp("pq", [96, 4096], BF16, isOutput=False)       # pred2 blocked [3c*32r, 128b*32col]
    tq_d = dp("tq", [96, 4096], BF16, isOutput=False)       # target blocked
    w27_d = dp("w27", [27, 64], BF16, isOutput=False)
    w12p_d = dp("w12p", [3, 128, 64], BF16, isOutput=False)
    w12s_d = dp("w12s", [3, 64, 64], BF16, isOutput=False)
    w21p_d = dp("w21p", [3, 128, 128], BF16, isOutput=False)
    w21s_d = dp("w21s", [3, 64, 128], BF16, isOutput=False)
    w22_d = dp("w22", [9, 128, 128], BF16, isOutput=False)
    w5_d = dp("w5", [9, 128, 256], BF16, isOutput=False)
    dftr_d = dp("dftr", [80, 64], BF16, isOutput=False)
    dfti_d = dp("dfti", [80, 64], BF16, isOutput=False)
    b1_d = dp("b1", [64, 1], F32, isOutput=False)
    b2_d = dp("b2", [64, 1], F32, isOutput=False)
    b3_d = dp("b3", [128, 1], F32, isOutput=False)
    b4_d = dp("b4", [128, 1], F32, isOutput=False)
    b5bc_d = dp("b5bc", [80, 256], F32, isOutput=False)
    o_d = dp("o", [1, 2], F32, isOutput=True)

    with TileContext(nc) as tc:
        with (
            tc.tile_pool(name="const", bufs=1) as cp,
            tc.tile_pool(name="xin", bufs=2) as xp,
            tc.tile_pool(name="mid", bufs=1) as mp,
            tc.tile_pool(name="hph", bufs=2) as hp,
            tc.tile_pool(name="tmp", bufs=1) as tp,
            tc.tile_pool(name="acc", bufs=1) as ac,
            tc.tile_pool(name="psa", bufs=2, space="PSUM") as pap,   # [64,504] 1 bank
            tc.tile_pool(name="psb", bufs=2, space="PSUM") as pbp,   # [128,1536] 3 banks
            tc.tile_pool(name="psd", bufs=1, space="PSUM") as pdp,   # pr/pi 1 bank each
        ):
            # ---- constants ----
            w27t = cp.tile([27, 64], BF16)
            nc.sync.dma_start(w27t[:, :], w27_d[:, :])
            w12pt = cp.tile([128, 3, 64], BF16)
            nc.sync.dma_start(w12pt[:, :, :], w12p_d[:, :, :].rearrange("a p m -> p a m"))
            w12st = cp.tile([64, 3, 64], BF16)
            nc.sync.dma_start(w12st[:, :, :], w12s_d[:, :, :].rearrange("a p m -> p a m"))
            w21pt = cp.tile([128, 3, 128], BF16)
            nc.sync.dma_start(w21pt[:, :, :], w21p_d[:, :, :].rearrange("a p m -> p a m"))
            w21st = cp.tile([64, 3, 128], BF16)
            nc.sync.dma_start(w21st[:, :, :], w21s_d[:, :, :].rearrange("a p m -> p a m"))
            w22t = cp.tile([128, 9, 128], BF16)
            nc.sync.dma_start(w22t[:, :, :], w22_d[:, :, :].rearrange("a p m -> p a m"))
            w5t = cp.tile([128, 9, 256], BF16)
            nc.sync.dma_start(w5t[:, :, :], w5_d[:, :, :].rearrange("a p m -> p a m"))
            dftrt = cp.tile([80, 64], BF16)
            nc.sync.dma_start(dftrt[:, :], dftr_d[:, :])
            dftit = cp.tile([80, 64], BF16)
            nc.sync.dma_start(dftit[:, :], dfti_d[:, :])
            b1t = cp.tile([64, 1], F32)
            nc.sync.dma_start(b1t[:, :], b1_d[:, :])
            b2t = cp.tile([64, 1], F32)
            nc.sync.dma_start(b2t[:, :], b2_d[:, :])
            b3t = cp.tile([128, 1], F32)
            nc.sync.dma_start(b3t[:, :], b3_d[:, :])
            b4t = cp.tile([128, 1], F32)
            nc.sync.dma_start(b4t[:, :], b4_d[:, :])
            b5bct = cp.tile([80, 256], F32)
            nc.sync.dma_start(b5bct[:, :], b5bc_d[:, :])
            ones64t = cp.tile([64, 1], F32)
            nc.gpsimd.memset(ones64t[:, :], 1.0)
            ones96t = cp.tile([96, 1], F32)
            nc.gpsimd.memset(ones96t[:, :], 1.0)

            # ---- accumulators ----
            STATS = ac.tile([1, NITER * 24], F32)   # [0, j*24 + stat*8 + t]
            L1S = ac.tile([96, 128], F32)           # per-block |p2-tg| partial sums

            # ---- persistent padded mid tiles (borders zeroed once) ----
            Y1PD = ac.tile([128, NB, 1160], BF16)
            Y2PD = ac.tile([128, NB * 324 + 40], BF16)
            Y3P = ac.tile([128, NB * 324 + 40], BF16)
            Y4P = ac.tile([128, NB, 104], BF16)
            nc.gpsimd.memset(Y1PD[:, :, 0:34], 0.0)
            nc.gpsimd.memset(Y1PD[:, :, 1122:1160], 0.0)
            nc.gpsimd.memset(
                Y1PD[:, :, 0:1156].rearrange(
                    "p b (r c) -> p b r c", c=34)[:, :, :, 0:34:33], 0.0)
            for TT in (Y2PD, Y3P):
                TV = TT[:, 0:NB * 324].rearrange("p (b f) -> p b f", f=324)
                nc.gpsimd.memset(TV[:, :, 0:18], 0.0)
                nc.gpsimd.memset(TV[:, :, 306:324], 0.0)
                nc.gpsimd.memset(TV.rearrange(
                    "p b (r c) -> p b r c", c=18)[:, :, :, 0:18:17], 0.0)
                nc.gpsimd.memset(TT[:, NB * 324:NB * 324 + 40], 0.0)
            nc.gpsimd.memset(Y4P[:, :, 0:10], 0.0)
            nc.gpsimd.memset(Y4P[:, :, 90:104], 0.0)
            nc.gpsimd.memset(
                Y4P[:, :, 0:100].rearrange(
                    "p b (r c) -> p b r c", c=10)[:, :, :, 0:10:9], 0.0)

            # ---- main loop ----
            for j in range(NITER):
                b0 = j * NB
                Ps = []
                for s, src_d in ((0, xa_d), (1, xb_d)):
                    # im2col: 9 shifted DMAs -> X27 [27, NB, 1224] (36-pitch planes)
                    X27 = xp.tile([27, NB, 1224], BF16, tag="x27")
                    for k in range(9):
                        dy, dx = k // 3, k % 3
                        off = dy * 36 + dx
                        nc.sync.dma_start(
                            X27[3 * k:3 * k + 3, :, 0:1222],
                            src_d[:, b0:b0 + NB, off:off + 1222])

                    # conv1_1 -> Y1PD (persistent, borders pre-zeroed)
                    for t in range(NB):
                        for r0, nr in ((0, 14), (14, 14), (28, 4)):  # row chunks
                            pA = pap.tile([64, 504], F32, tag="pa")
                            nc.tensor.matmul(
                                pA[:, 0:nr * 36], w27t[:, :],
                                X27[:, t, r0 * 36:(r0 + nr) * 36],
                                start=True, stop=True)
                            dst = Y1PD[0:64, t, 35 + r0 * 34:35 + (r0 + nr) * 34].rearrange(
                                "p (r c) -> p r c", c=34)[:, :, 0:32]
                            nc.scalar.activation(
                                dst,
                                pA[:, 0:nr * 36].rearrange(
                                    "p (r c) -> p r c", c=36)[:, :, 0:32],
                                ACTF.Relu, bias=b1t[:, 0:1])
                    nc.sync.dma_start(
                        Y1PD[64:128, :, 0:1155], Y1PD[0:64, :, 1:1156])

                    # conv1_2 (+bias2+relu+pool1) -> Y2PD (persistent)
                    for t in range(NB):
                        for r0, nr in ((0, 14), (14, 14), (28, 4)):  # row chunks
                            pB = pap.tile([64, 504], F32, tag="pa")
                            nw = nr * 34
                            base = r0 * 34
                            for dy in range(3):
                                nc.tensor.matmul(
                                    pB[:, 0:nw], w12pt[:, dy, :],
                                    Y1PD[:, t, base + dy * 34:base + dy * 34 + nw],
                                    start=(dy == 0), stop=False)
                            for dy in range(3):
                                nc.tensor.matmul(
                                    pB[:, 0:nw], w12st[:, dy, :],
                                    Y1PD[0:64, t, base + dy * 34 + 2:base + dy * 34 + 2 + nw],
                                    start=False, stop=(dy == 2))
                            pool_in = pB[:, 0:nw].rearrange(
                                "p (u v) -> p u v", v=34)[:, :, 0:32].rearrange(
                                "p (r dr) (c dc) -> p r c dr dc", dr=2, dc=2)
                            t3 = tp.tile([64, 7, 16], F32, tag="pl")
                            nc.vector.tensor_reduce(t3[:, 0:nr // 2, :], pool_in,
                                                    axis=AXL.XY, op=ALU.max)
                            dst = Y2PD[0:64, t * 324 + 19 + (r0 // 2) * 18:t * 324 + 19 + (r0 // 2 + nr // 2) * 18].rearrange(
                                "p (r c) -> p r c", c=18)[:, :, 0:16]
                            nc.vector.tensor_scalar(
                                out=dst, in0=t3[:, 0:nr // 2, :], scalar1=b2t[:, 0:1],
                                scalar2=0.0, op0=ALU.add, op1=ALU.max)
                    nc.sync.dma_start(
                        Y2PD[64:128, 0:NB * 324 + 39], Y2PD[0:64, 1:NB * 324 + 40])

                    # conv2_1 -> Y3P (tall: 4 blocks per psum mega)
                    for g in range(NB // 4):
                        pC = pbp.tile([128, 1536], F32, tag="pb")
                        for c0, nn_ in ((0, 512), (512, 512), (1024, 272)):
                            base = g * 1296 + c0
                            for dy in range(3):
                                nc.tensor.matmul(
                                    pC[:, c0:c0 + nn_], w21pt[:, dy, :],
                                    Y2PD[:, base + dy * 18:base + dy * 18 + nn_],
                                    start=(dy == 0), stop=False)
                            for dy in range(3):
                                nc.tensor.matmul(
                                    pC[:, c0:c0 + nn_], w21st[:, dy, :],
                                    Y2PD[0:64, base + dy * 18 + 2:base + dy * 18 + 2 + nn_],
                                    start=False, stop=(dy == 2))
                        for i in range(4):
                            dst = Y3P[:, (4 * g + i) * 324 + 19:(4 * g + i) * 324 + 19 + 16 * 18].rearrange(
                                "p (r c) -> p r c", c=18)[:, :, 0:16]
                            nc.scalar.activation(
                                dst,
                                pC[:, i * 324:i * 324 + 288].rearrange(
                                    "p (r c) -> p r c", c=18)[:, :, 0:16],
                                ACTF.Relu, bias=b3t[:, 0:1])

                    # conv2_2 (+bias4+relu+pool2) -> Y4P (tall megas)
                    for g in range(NB // 4):
                        pD = pbp.tile([128, 1536], F32, tag="pb")
                        for c0, nn_ in ((0, 512), (512, 512), (1024, 272)):
                            base = g * 1296 + c0
                            for k in range(9):
                                dy, dx = k // 3, k % 3
                                nc.tensor.matmul(
                                    pD[:, c0:c0 + nn_], w22t[:, k, :],
                                    Y3P[:, base + dy * 18 + dx:base + dy * 18 + dx + nn_],
                                    start=(k == 0), stop=(k == 8))
                        for i in range(4):
                            t = 4 * g + i
                            pool_in = pD[:, i * 324:i * 324 + 288].rearrange(
                                "p (u v) -> p u v", v=18)[:, :, 0:16].rearrange(
                                "p (r dr) (c dc) -> p r c dr dc", dr=2, dc=2)
                            t3 = tp.tile([128, 8, 8], F32, tag="ql")
                            nc.vector.tensor_reduce(t3[:, :, :], pool_in,
                                                    axis=AXL.XY, op=ALU.max)
                            dst = Y4P[:, t, 11:11 + 8 * 10].rearrange(
                                "p (r c) -> p r c", c=10)[:, :, 0:8]
                            nc.vector.tensor_scalar(
                                out=dst, in0=t3[:, :, :], scalar1=b4t[:, 0:1],
                                scalar2=0.0, op0=ALU.add, op1=ALU.max)

                    # conv3_1 transposed (data stationary): out [80sp(10-pitch), 256ch]
                    # + b5 broadcast -> H80; junk pitch rows killed later by dft rows=0
                    H80 = hp.tile([80, NB, 256], BF16, tag="h")
                    for t in range(NB):
                        pH = pap.tile([80, 256], F32, tag="pa")
                        for k in range(9):
                            dy, dx = k // 3, k % 3
                            nc.tensor.matmul(
                                pH[:, :], Y4P[:, t, dy * 10 + dx:dy * 10 + dx + 80],
                                w5t[:, k, :], start=(k == 0), stop=(k == 8))
                        nc.vector.tensor_tensor(out=H80[:, t, :], in0=pH[:, :],
                                                in1=b5bct[:, :], op=ALU.add)

                    # DFT (8x8, real+imag), batched: chunks of 512 over flat blocks*ch
                    PRb = tp.tile([64, NB, 256], F32, tag="prb")
                    PIb = tp.tile([64, NB, 256], F32, tag="pib")
                    H80f = H80[:, :, :].rearrange("p a b -> p (a b)")
                    PRbf = PRb[:, :, :].rearrange("p a b -> p (a b)")
                    PIbf = PIb[:, :, :].rearrange("p a b -> p (a b)")
                    for c0 in range(0, NB * 256, 512):
                        pR = pap.tile([64, 512], F32, tag="pa")
                        nc.tensor.matmul(pR[:, :], dftrt[:, :], H80f[:, c0:c0 + 512],
                                         start=True, stop=True)
                        pI = pap.tile([64, 512], F32, tag="pa")
                        nc.tensor.matmul(pI[:, :], dftit[:, :], H80f[:, c0:c0 + 512],
                                         start=True, stop=True)
                        nc.scalar.activation(PRbf[:, c0:c0 + 512], pR[:, :], ACTF.Copy)
                        nc.vector.tensor_copy(PIbf[:, c0:c0 + 512], pI[:, :])

                    PRf = PRb[:, :, :].rearrange("p a b -> p (a b)")
                    PIf = PIb[:, :, :].rearrange("p a b -> p (a b)")
                    rinv = tp.tile([64, NB * 256], F32, tag="phD")
                    nc.vector.reciprocal_approx_fast(rinv[:, :], PRf)
                    tq_t = tp.tile([64, NB * 256], F32, tag="phA")
                    nc.vector.tensor_tensor(out=tq_t[:, :], in0=PIf, in1=rinv[:, :], op=ALU.mult)
                    ta = tp.tile([64, NB * 256], F32, tag="phB")
                    nc.scalar.activation(ta[:, :], tq_t[:, :], ACTF.Arctan)
                    tneg = tp.tile([64, NB * 256], F32, tag="phC")
                    nc.vector.tensor_scalar(out=tneg[:, :], in0=PRf,
                                            scalar1=0.0, scalar2=None, op0=ALU.is_lt)
                    tt_ = tp.tile([64, NB * 256], F32, tag="phA")
                    nc.vector.scalar_tensor_tensor(
                        out=tt_[:, :], in0=PIf, scalar=0.0, in1=tneg[:, :],
                        op0=ALU.is_ge, op1=ALU.mult)
                    tu = tp.tile([64, NB * 256], F32, tag="phD")
                    nc.vector.tensor_tensor(out=tu[:, :], in0=tneg[:, :],
                                            in1=tt_[:, :], op=ALU.subtract)
                    tv = tp.tile([64, NB * 256], F32, tag="phC")
                    nc.vector.tensor_tensor(out=tv[:, :], in0=tt_[:, :],
                                            in1=tu[:, :], op=ALU.subtract)
                    P = hp.tile([64, NB * 256], F32, tag=f"p{s}")
                    nc.vector.scalar_tensor_tensor(
                        out=P[:, :], in0=tv[:, :], scalar=PI, in1=ta[:, :],
                        op0=ALU.mult, op1=ALU.add)
                    Ps.append(P)

                # cosine-sim stats: per-block sums of p1*p2, p1^2, p2^2
                R3 = tp.tile([64, 3, NB], F32, tag="r3")
                pm = tp.tile([64, NB * 256], F32, tag="phD")
                for si, (ia, ib) in enumerate(((0, 1), (0, 0), (1, 1))):
                    nc.vector.tensor_tensor(out=pm[:, :], in0=Ps[ia][:, :],
                                            in1=Ps[ib][:, :], op=ALU.mult)
                    nc.vector.tensor_reduce(
                        R3[:, si, :], pm[:, :].rearrange("p (a b) -> p a b", a=NB),
                        axis=AXL.X, op=ALU.add)
                pS = pap.tile([1, 3 * NB], F32, tag="pa")
                nc.tensor.matmul(pS[:, :], ones64t[:, :],
                                 R3[:, :, :].rearrange("p a b -> p (a b)"),
                                 start=True, stop=True)
                nc.vector.tensor_copy(STATS[:, j * 24:(j + 1) * 24], pS[:, :])

                # L1 partial sums for this iteration's 8 blocks
                pqc = xp.tile([96, 256], BF16, tag="pqc")
                nc.sync.dma_start(pqc[:, :], pq_d[:, j * 256:(j + 1) * 256])
                tqc = xp.tile([96, 256], BF16, tag="tqc")
                nc.sync.dma_start(tqc[:, :], tq_d[:, j * 256:(j + 1) * 256])
                dl = tp.tile([96, 256], F32, tag="dl")
                nc.vector.tensor_tensor(out=dl[:, :], in0=pqc[:, :],
                                        in1=tqc[:, :], op=ALU.subtract)
                nc.vector.tensor_reduce(
                    L1S[:, j * 8:(j + 1) * 8],
                    dl[:, :].rearrange("p (a b) -> p a b", a=8),
                    axis=AXL.X, op=ALU.add, apply_absolute_value=True)

            # ---- epilogue (all tiny, partition 0) ----
            pL = pap.tile([1, 128], F32, tag="pa")
            nc.tensor.matmul(pL[:, :], ones96t[:, :], L1S[:, :], start=True, stop=True)
            R1 = tp.tile([1, 128], F32, tag="r1")
            nc.vector.tensor_copy(R1[:, :], pL[:, :])
            SV = STATS[:, :].rearrange("p (j s t) -> p j s t", s=3, t=8)
            NUM = SV[:, :, 0, :]
            N1 = SV[:, :, 1, :]
            N2 = SV[:, :, 2, :]
            W_ = tp.tile([1, 16, 8], F32, tag="w_")
            nc.vector.tensor_tensor(out=W_[:, :, :], in0=N1, in1=N2, op=ALU.mult)
            S2 = tp.tile([1, 16, 8], F32, tag="s2")
            nc.vector.tensor_tensor(out=S2[:, :, :], in0=NUM, in1=NUM, op=ALU.mult)
            C2 = tp.tile([1, 16, 8], F32, tag="c2")
            nc.vector.scalar_tensor_tensor(
                out=C2[:, :, :], in0=S2[:, :, :], scalar=25.0, in1=W_[:, :, :],
                op0=ALU.mult, op1=ALU.is_ge)
            M1 = tp.tile([1, 16, 8], F32, tag="m1")
            nc.vector.tensor_scalar(out=M1[:, :, :], in0=NUM, scalar1=0.0,
                                    scalar2=None, op0=ALU.is_gt)
            MASK = tp.tile([1, 128], F32, tag="mask")
            nc.vector.tensor_tensor(
                out=MASK[:, :].rearrange("p (a b) -> p a b", a=16),
                in0=C2[:, :, :], in1=M1[:, :, :], op=ALU.mult)
            PR_ = tp.tile([1, 128], F32, tag="pr_")
            nc.vector.tensor_tensor(out=PR_[:, :], in0=MASK[:, :],
                                    in1=R1[:, :], op=ALU.mult)
            OT = tp.tile([1, 2], F32, tag="ot")
            nc.vector.tensor_reduce(OT[:, 0:1], PR_[:, :], axis=AXL.X, op=ALU.add)
            nc.vector.tensor_reduce(OT[:, 1:2], MASK[:, :], axis=AXL.X, op=ALU.add)
            nc.sync.dma_start(o_d[:, :], OT[:, :])
    nc.compile()
    return nc


def _blocks(x, c):
    # [16, c, 256, 256] -> [1024, c, 32, 32], block = img*64 + by*8 + bx
    return (x.reshape(16, c, 8, 32, 8, 32).transpose(0, 2, 4, 1, 3, 5)
            .reshape(1024, c, 32, 32))


def kernel(pred1, pred2, target, w1, b1, w2, b2, w3, b3, w4, b4, w5, b5):
    import ml_dtypes
    from concourse.bass_utils import run_bass_kernel_spmd
    bf = ml_dtypes.bfloat16

    pred1 = np.asarray(pred1, np.float32)
    pred2 = np.asarray(pred2, np.float32)
    target = np.asarray(target, np.float32)
    w1, w2, w3, w4, w5 = (np.asarray(w, np.float32) for w in (w1, w2, w3, w4, w5))
    b1, b2, b3, b4, b5 = (np.asarray(b, np.float32) for b in (b1, b2, b3, b4, b5))

    def taps_T(w):  # [O, I, 3, 3] -> [9, I, O]
        return np.ascontiguousarray(
            w.transpose(2, 3, 1, 0).reshape(9, w.shape[1], w.shape[0]))

    w1T, w2T, w3T, w4T, w5T = map(taps_T, (w1, w2, w3, w4, w5))
    w27 = np.ascontiguousarray(w1T.reshape(27, 64)).astype(bf)
    w12p = np.stack([np.concatenate([w2T[3 * dy], w2T[3 * dy + 1]], 0)
                     for dy in range(3)]).astype(bf)
    w12s = np.stack([w2T[3 * dy + 2] for dy in range(3)]).astype(bf)
    w21p = np.stack([np.concatenate([w3T[3 * dy], w3T[3 * dy + 1]], 0)
                     for dy in range(3)]).astype(bf)
    w21s = np.stack([w3T[3 * dy + 2] for dy in range(3)]).astype(bf)
    w22 = w4T.astype(bf)
    w5p = w5T.astype(bf)

    idx = np.arange(8)
    s_in = (idx[:, None, None, None] * idx[None, None, :, None]
            + idx[None, :, None, None] * idx[None, None, None, :])  # [ri,ci,ro,co]
    theta = 2.0 * np.pi * (s_in % 8) / 8.0
    M_r = np.cos(theta).reshape(64, 64).astype(np.float32)
    M_i = (-np.sin(theta)).reshape(64, 64).astype(np.float32)
    M_r[np.abs(M_r) < 1e-6] = 0.0
    M_i[np.abs(M_i) < 1e-6] = 0.0
    # 10-pitch layout: row p = r*10 + c holds spatial (r,c); pitch cols 8,9 = 0
    dftr = np.zeros((80, 64), np.float32)
    dfti = np.zeros((80, 64), np.float32)
    for r in range(8):
        dftr[r * 10:r * 10 + 8, :] = M_r[r * 8:(r + 1) * 8, :]
        dfti[r * 10:r * 10 + 8, :] = M_i[r * 8:(r + 1) * 8, :]
    b5bc = np.tile(b5.reshape(1, 256), (80, 1)).astype(np.float32)

    shared = dict(
        w27=w27, w12p=w12p, w12s=w12s, w21p=w21p, w21s=w21s, w22=w22, w5=w5p,
        dftr=dftr.astype(bf), dfti=dfti.astype(bf),
        b1=np.ascontiguousarray(b1.reshape(64, 1)),
        b2=np.ascontiguousarray(b2.reshape(64, 1)),
        b3=np.ascontiguousarray(b3.reshape(128, 1)),
        b4=np.ascontiguousarray(b4.reshape(128, 1)), b5bc=b5bc)

    p1n = ((pred1 - MEAN) / STD).astype(np.float32)
    tgn = ((target - MEAN) / STD).astype(np.float32)
    bl1 = _blocks(p1n, 3)
    blt = _blocks(tgn, 3)
    blp2 = _blocks(pred2, 3)
    bltg = _blocks(target, 3)

    def pad36(blk):  # [128, 3, 32, 32] -> [3, 128, 1296] bf16
        out = np.zeros((3, 128, 36, 36), np.float32)
        out[:, :, 2:34, 2:34] = blk.transpose(1, 0, 2, 3)
        return out.reshape(3, 128, 1296).astype(bf)

    def l1fmt(blk):  # [128, 3, 32, 32] -> [96, 4096] bf16
        return np.ascontiguousarray(
            blk.transpose(1, 2, 0, 3).reshape(96, 128 * 32)).astype(bf)

    in_maps = []
    for c in range(N_CORES):
        s = slice(c * 128, (c + 1) * 128)
        in_maps.append(dict(
            xa=pad36(bl1[s]), xb=pad36(blt[s]),
            pq=l1fmt(blp2[s]), tq=l1fmt(bltg[s]), **shared))

    if "nc" not in _COMPILED:
        _COMPILED["nc"] = _build_nc()
    nc = _COMPILED["nc"]

    import time as _time
    t0 = _time.perf_counter()
    try:
        res = run_bass_kernel_spmd(nc, in_maps, list(range(N_CORES)), trace=PROFILE)
    except ModuleNotFoundError:
        res = run_bass_kernel_spmd(nc, in_maps, list(range(N_CORES)))
    global LAST_EXEC_NS, LAST_TRACE_PATH
    LAST_EXEC_NS = int((_time.perf_counter() - t0) * 1e9)
    if res.exec_time_ns:
        LAST_EXEC_NS = int(res.exec_time_ns)
    if res.instructions_and_trace:
        LAST_TRACE_PATH = res.instructions_and_trace[1]

    l1 = 0.0
    cnt = 0.0
    for c in range(N_CORES):
        o = res.results[c]["o"]
        l1 += float(o[0, 0])
        cnt += float(o[0, 1])
    out = np.float32(l1 / (cnt * 1024.0 + 1e-6))
    return np.array(out, dtype=np.float32)


# revision 4
# speedup vs baseline: 1.2545x; 1.0026x over previous
"""nn_LphaLoss kernel — full pipeline on 8 TRN2 NeuronCores via Bass/Tile.

Per core (2 of 16 images = 128 blocks/stream): VGG conv3_1 features for
pred1/target 32x32 blocks (bf16 matmuls, contiguous flat-window rhs,
K-packed taps), 8x8 2D-DFT via [80,64] matmuls (junk pitch rows zeroed in
the DFT matrix), FFT-phase via atan2 on device, cosine similarity -> block
mask, masked L1 partial sums. Host sums 8 scalar pairs and divides.

kernel(**inputs) takes FULL inputs, returns the FULL (scalar) output.
"""
import numpy as np

BS = 32
N_CORES = 8
NB = 8            # blocks per stream per iteration
NITER = 16        # 128 blocks per stream / NB
MEAN = np.array([0.485, 0.456, 0.406], dtype=np.float32).reshape(1, 3, 1, 1)
STD = np.array([0.229, 0.224, 0.225], dtype=np.float32).reshape(1, 3, 1, 1)

_COMPILED = {}
LAST_EXEC_NS = None
PROFILE = False          # when True, run with NTFF tracing; LAST_EXEC_NS = device time
LAST_TRACE_PATH = None


def _build_nc():
    import concourse.mybir as mybir
    from concourse import bacc
    from concourse.tile import TileContext

    F32 = mybir.dt.float32
    BF16 = mybir.dt.bfloat16
    ALU = mybir.AluOpType
    ACTF = mybir.ActivationFunctionType
    AXL = mybir.AxisListType
    PI = float(np.pi)

    nc = bacc.Bacc("TRN2", target_bir_lowering=False)
    dp = nc.declare_dram_parameter
    xa_d = dp("xa", [3, 128, 1296], BF16, isOutput=False)   # pred1 blocks, 36x36 padded
    xb_d = dp("xb", [3, 128, 1296], BF16, isOutput=False)   # target blocks
    pq_d = dp("pq", [96, 4096], BF16, isOutput=False)       # pred2 blocked [3c*32r, 128b*32col]
    tq_d = dp("tq", [96, 4096], BF16, isOutput=False)       # target blocked
    w27_d = dp("w27", [27, 64], BF16, isOutput=False)
    w12p_d = dp("w12p", [3, 128, 64], BF16, isOutput=False)
    w12s_d = dp("w12s", [3, 64, 64], BF16, isOutput=False)
    w21p_d = dp("w21p", [3, 128, 128], BF16, isOutput=False)
    w21s_d = dp("w21s", [3, 64, 128], BF16, isOutput=False)
    w22_d = dp("w22", [9, 128, 128], BF16, isOutput=False)
    w5_d = dp("w5", [9, 128, 256], BF16, isOutput=False)
    dftr_d = dp("dftr", [80, 64], BF16, isOutput=False)
    dfti_d = dp("dfti", [80, 64], BF16, isOutput=False)
    b1_d = dp("b1", [64, 1], F32, isOutput=False)
    b2_d = dp("b2", [64, 1], F32, isOutput=False)
    b3_d = dp("b3", [128, 1], F32, isOutput=False)
    b4_d = dp("b4", [128, 1], F32, isOutput=False)
    b5bc_d = dp("b5bc", [80, 256], F32, isOutput=False)
    o_d = dp("o", [1, 2], F32, isOutput=True)

    with TileContext(nc) as tc:
        with (
            tc.tile_pool(name="const", bufs=1) as cp,
            tc.tile_pool(name="xin", bufs=2) as xp,
            tc.tile_pool(name="mid", bufs=1) as mp,
            tc.tile_pool(name="hph", bufs=2) as hp,
            tc.tile_pool(name="tmp", bufs=1) as tp,
            tc.tile_pool(name="acc", bufs=1) as ac,
            tc.tile_pool(name="psa", bufs=2, space="PSUM") as pap,   # [64,504] 1 bank
            tc.tile_pool(name="psb", bufs=2, space="PSUM") as pbp,   # [128,1536] 3 banks
            tc.tile_pool(name="psd", bufs=1, space="PSUM") as pdp,   # pr/pi 1 bank each
        ):
            # ---- constants ----
            w27t = cp.tile([27, 64], BF16)
            nc.sync.dma_start(w27t[:, :], w27_d[:, :])
            w12pt = cp.tile([128, 3, 64], BF16)
            nc.sync.dma_start(w12pt[:, :, :], w12p_d[:, :, :].rearrange("a p m -> p a m"))
            w12st = cp.tile([64, 3, 64], BF16)
            nc.sync.dma_start(w12st[:, :, :], w12s_d[:, :, :].rearrange("a p m -> p a m"))
            w21pt = cp.tile([128, 3, 128], BF16)
            nc.sync.dma_start(w21pt[:, :, :], w21p_d[:, :, :].rearrange("a p m -> p a m"))
            w21st = cp.tile([64, 3, 128], BF16)
            nc.sync.dma_start(w21st[:, :, :], w21s_d[:, :, :].rearrange("a p m -> p a m"))
            w22t = cp.tile([128, 9, 128], BF16)
            nc.sync.dma_start(w22t[:, :, :], w22_d[:, :, :].rearrange("a p m -> p a m"))
            w5t = cp.tile([128, 9, 256], BF16)
            nc.sync.dma_start(w5t[:, :, :], w5_d[:, :, :].rearrange("a p m -> p a m"))
            dftrt = cp.tile([80, 64], BF16)
            nc.sync.dma_start(dftrt[:, :], dftr_d[:, :])
            dftit = cp.tile([80, 64], BF16)
            nc.sync.dma_start(dftit[:, :], dfti_d[:, :])
            b1t = cp.tile([64, 1], F32)
            nc.sync.dma_start(b1t[:, :], b1_d[:, :])
            b2t = cp.tile([64, 1], F32)
            nc.sync.dma_start(b2t[:, :], b2_d[:, :])
            b3t = cp.tile([128, 1], F32)
            nc.sync.dma_start(b3t[:, :], b3_d[:, :])
            b4t = cp.tile([128, 1], F32)
            nc.sync.dma_start(b4t[:, :], b4_d[:, :])
            b5bct = cp.tile([80, 256], F32)
            nc.sync.dma_start(b5bct[:, :], b5bc_d[:, :])
            ones64t = cp.tile([64, 1], F32)
            nc.gpsimd.memset(ones64t[:, :], 1.0)
            ones96t = cp.tile([96, 1], F32)
            nc.gpsimd.memset(ones96t[:, :], 1.0)

            # ---- accumulators ----
            STATS = ac.tile([1, NITER * 24], F32)   # [0, j*24 + stat*8 + t]
            L1S = ac.tile([96, 128], F32)           # per-block |p2-tg| partial sums

            # ---- persistent padded mid tiles (borders zeroed once) ----
            Y1PD = ac.tile([128, NB * 1156 + 80], BF16)
            Y2PD = ac.tile([128, NB * 324 + 40], BF16)
            Y3P = ac.tile([128, NB * 324 + 40], BF16)
            Y4P = ac.tile([128, NB, 104], BF16)
            Y1V = Y1PD[:, 0:NB * 1156].rearrange("p (b f) -> p b f", f=1156)
            nc.gpsimd.memset(Y1V[:, :, 0:34], 0.0)
            nc.gpsimd.memset(Y1V[:, :, 1122:1156], 0.0)
            nc.gpsimd.memset(Y1V.rearrange(
                "p b (r c) -> p b r c", c=34)[:, :, :, 0:34:33], 0.0)
            nc.gpsimd.memset(Y1PD[:, NB * 1156:NB * 1156 + 80], 0.0)
            for TT in (Y2PD, Y3P):
                TV = TT[:, 0:NB * 324].rearrange("p (b f) -> p b f", f=324)
                nc.gpsimd.memset(TV[:, :, 0:18], 0.0)
                nc.gpsimd.memset(TV[:, :, 306:324], 0.0)
                nc.gpsimd.memset(TV.rearrange(
                    "p b (r c) -> p b r c", c=18)[:, :, :, 0:18:17], 0.0)
                nc.gpsimd.memset(TT[:, NB * 324:NB * 324 + 40], 0.0)
            nc.gpsimd.memset(Y4P[:, :, 0:10], 0.0)
            nc.gpsimd.memset(Y4P[:, :, 90:104], 0.0)
            nc.gpsimd.memset(
                Y4P[:, :, 0:100].rearrange(
                    "p b (r c) -> p b r c", c=10)[:, :, :, 0:10:9], 0.0)

            # ---- main loop ----
            for j in range(NITER):
                b0 = j * NB
                Ps = []
                for s, src_d in ((0, xa_d), (1, xb_d)):
                    # im2col: 9 shifted DMAs -> X27 [27, NB, 1224] (36-pitch planes)
                    X27 = xp.tile([27, NB, 1224], BF16, tag="x27")
                    for k in range(9):
                        dy, dx = k // 3, k % 3
                        off = dy * 36 + dx
                        nc.sync.dma_start(
                            X27[3 * k:3 * k + 3, :, 0:1222],
                            src_d[:, b0:b0 + NB, off:off + 1222])

                    # conv1_1 -> Y1PD (persistent, borders pre-zeroed)
                    for t in range(NB):
                        for r0, nr in ((0, 14), (14, 14), (28, 4)):  # row chunks
                            pA = pap.tile([64, 504], F32, tag="pa")
                            nc.tensor.matmul(
                                pA[:, 0:nr * 36], w27t[:, :],
                                X27[:, t, r0 * 36:(r0 + nr) * 36],
                                start=True, stop=True)
                            dst = Y1PD[0:64, t * 1156 + 35 + r0 * 34:t * 1156 + 35 + (r0 + nr) * 34].rearrange(
                                "p (r c) -> p r c", c=34)[:, :, 0:32]
                            nc.scalar.activation(
                                dst,
                                pA[:, 0:nr * 36].rearrange(
                                    "p (r c) -> p r c", c=36)[:, :, 0:32],
                                ACTF.Relu, bias=b1t[:, 0:1])
                    nc.sync.dma_start(
                        Y1PD[64:128, 0:NB * 1156 + 79], Y1PD[0:64, 1:NB * 1156 + 80])

                    # conv1_2: tall flat chunks into 3-bank megas, pool per block
                    Y12 = tp.tile([64, NB * 1156], BF16, tag="y12")
                    TOT = NB * 1156
                    chunks = [(c0, min(512, TOT - c0)) for c0 in range(0, TOT, 512)]
                    for m0 in range(0, len(chunks), 3):
                        mega = chunks[m0:m0 + 3]
                        pB = pbp.tile([64, 1536], F32, tag="pb")
                        for ci, (c0, nn_) in enumerate(mega):
                            for dy in range(3):
                                nc.tensor.matmul(
                                    pB[:, ci * 512:ci * 512 + nn_], w12pt[:, dy, :],
                                    Y1PD[:, c0 + dy * 34:c0 + dy * 34 + nn_],
                                    start=(dy == 0), stop=False)
                            for dy in range(3):
                                nc.tensor.matmul(
                                    pB[:, ci * 512:ci * 512 + nn_], w12st[:, dy, :],
                                    Y1PD[0:64, c0 + dy * 34 + 2:c0 + dy * 34 + 2 + nn_],
                                    start=False, stop=(dy == 2))
                        for ci, (c0, nn_) in enumerate(mega):
                            nc.vector.tensor_copy(Y12[:, c0:c0 + nn_],
                                                  pB[:, ci * 512:ci * 512 + nn_])
                    for t in range(NB):
                        pool_in = Y12[:, t * 1156:t * 1156 + 32 * 34].rearrange(
                            "p (u v) -> p u v", v=34)[:, :, 0:32].rearrange(
                            "p (r dr) (c dc) -> p r c dr dc", dr=2, dc=2)
                        t3 = tp.tile([64, 16, 16], F32, tag="pl")
                        nc.vector.tensor_reduce(t3[:, :, :], pool_in,
                                                axis=AXL.XY, op=ALU.max)
                        dst = Y2PD[0:64, t * 324 + 19:t * 324 + 19 + 16 * 18].rearrange(
                            "p (r c) -> p r c", c=18)[:, :, 0:16]
                        nc.vector.tensor_scalar(
                            out=dst, in0=t3[:, :, :], scalar1=b2t[:, 0:1],
                            scalar2=0.0, op0=ALU.add, op1=ALU.max)
                    nc.sync.dma_start(
                        Y2PD[64:128, 0:NB * 324 + 39], Y2PD[0:64, 1:NB * 324 + 40])

                    # conv2_1 -> Y3P (tall: 4 blocks per psum mega)
                    for g in range(NB // 4):
                        pC = pbp.tile([128, 1536], F32, tag="pb")
                        for c0, nn_ in ((0, 512), (512, 512), (1024, 272)):
                            base = g * 1296 + c0
                            for dy in range(3):
                                nc.tensor.matmul(
                                    pC[:, c0:c0 + nn_], w21pt[:, dy, :],
                                    Y2PD[:, base + dy * 18:base + dy * 18 + nn_],
                                    start=(dy == 0), stop=False)
                            for dy in range(3):
                                nc.tensor.matmul(
                                    pC[:, c0:c0 + nn_], w21st[:, dy, :],
                                    Y2PD[0:64, base + dy * 18 + 2:base + dy * 18 + 2 + nn_],
                                    start=False, stop=(dy == 2))
                        for i in range(4):
                            dst = Y3P[:, (4 * g + i) * 324 + 19:(4 * g + i) * 324 + 19 + 16 * 18].rearrange(
                                "p (r c) -> p r c", c=18)[:, :, 0:16]
                            nc.scalar.activation(
                                dst,
                                pC[:, i * 324:i * 324 + 288].rearrange(
                                    "p (r c) -> p r c", c=18)[:, :, 0:16],
                                ACTF.Relu, bias=b3t[:, 0:1])

                    # conv2_2 (+bias4+relu+pool2) -> Y4P (tall megas)
                    for g in range(NB // 4):
                        pD = pbp.tile([128, 1536], F32, tag="pb")
                        for c0, nn_ in ((0, 512), (512, 512), (1024, 272)):
                            base = g * 1296 + c0
                            for k in range(9):
                                dy, dx = k // 3, k % 3
                                nc.tensor.matmul(
                                    pD[:, c0:c0 + nn_], w22t[:, k, :],
                                    Y3P[:, base + dy * 18 + dx:base + dy * 18 + dx + nn_],
                                    start=(k == 0), stop=(k == 8))
                        for i in range(4):
                            t = 4 * g + i
                            pool_in = pD[:, i * 324:i * 324 + 288].rearrange(
                                "p (u v) -> p u v", v=18)[:, :, 0:16].rearrange(
                                "p (r dr) (c dc) -> p r c dr dc", dr=2, dc=2)
                            t3 = tp.tile([128, 8, 8], F32, tag="ql")
                            nc.vector.tensor_reduce(t3[:, :, :], pool_in,
                                                    axis=AXL.XY, op=ALU.max)
                            dst = Y4P[:, t, 11:11 + 8 * 10].rearrange(
                                "p (r c) -> p r c", c=10)[:, :, 0:8]
                            nc.vector.tensor_scalar(
                                out=dst, in0=t3[:, :, :], scalar1=b4t[:, 0:1],
                                scalar2=0.0, op0=ALU.add, op1=ALU.max)

                    # conv3_1 transposed (data stationary): out [80sp(10-pitch), 256ch]
                    # + b5 broadcast -> H80; junk pitch rows killed later by dft rows=0
                    H80 = hp.tile([80, NB, 256], BF16, tag="h")
                    for t in range(NB):
                        pH = pap.tile([80, 256], F32, tag="pa")
                        for k in range(9):
                            dy, dx = k // 3, k % 3
                            nc.tensor.matmul(
                                pH[:, :], Y4P[:, t, dy * 10 + dx:dy * 10 + dx + 80],
                                w5t[:, k, :], start=(k == 0), stop=(k == 8))
                        nc.vector.tensor_tensor(out=H80[:, t, :], in0=pH[:, :],
                                                in1=b5bct[:, :], op=ALU.add)

                    # DFT (8x8, real+imag), batched: chunks of 512 over flat blocks*ch
                    PRb = tp.tile([64, NB, 256], F32, tag="prb")
                    PIb = tp.tile([64, NB, 256], F32, tag="pib")
                    H80f = H80[:, :, :].rearrange("p a b -> p (a b)")
                    PRbf = PRb[:, :, :].rearrange("p a b -> p (a b)")
                    PIbf = PIb[:, :, :].rearrange("p a b -> p (a b)")
                    for c0 in range(0, NB * 256, 512):
                        pR = pap.tile([64, 512], F32, tag="pa")
                        nc.tensor.matmul(pR[:, :], dftrt[:, :], H80f[:, c0:c0 + 512],
                                         start=True, stop=True)
                        pI = pap.tile([64, 512], F32, tag="pa")
                        nc.tensor.matmul(pI[:, :], dftit[:, :], H80f[:, c0:c0 + 512],
                                         start=True, stop=True)
                        nc.scalar.activation(PRbf[:, c0:c0 + 512], pR[:, :], ACTF.Copy)
                        nc.vector.tensor_copy(PIbf[:, c0:c0 + 512], pI[:, :])

                    PRf = PRb[:, :, :].rearrange("p a b -> p (a b)")
                    PIf = PIb[:, :, :].rearrange("p a b -> p (a b)")
                    rinv = tp.tile([64, NB * 256], F32, tag="phD")
                    nc.vector.reciprocal_approx_fast(rinv[:, :], PRf)
                    tq_t = tp.tile([64, NB * 256], F32, tag="phA")
                    nc.vector.tensor_tensor(out=tq_t[:, :], in0=PIf, in1=rinv[:, :], op=ALU.mult)
                    ta = tp.tile([64, NB * 256], F32, tag="phB")
                    nc.scalar.activation(ta[:, :], tq_t[:, :], ACTF.Arctan)
                    tneg = tp.tile([64, NB * 256], F32, tag="phC")
                    nc.vector.tensor_scalar(out=tneg[:, :], in0=PRf,
                                            scalar1=0.0, scalar2=None, op0=ALU.is_lt)
                    tt_ = tp.tile([64, NB * 256], F32, tag="phA")
                    nc.vector.scalar_tensor_tensor(
                        out=tt_[:, :], in0=PIf, scalar=0.0, in1=tneg[:, :],
                        op0=ALU.is_ge, op1=ALU.mult)
                    tu = tp.tile([64, NB * 256], F32, tag="phD")
                    nc.vector.tensor_tensor(out=tu[:, :], in0=tneg[:, :],
                                            in1=tt_[:, :], op=ALU.subtract)
                    tv = tp.tile([64, NB * 256], F32, tag="phC")
                    nc.vector.tensor_tensor(out=tv[:, :], in0=tt_[:, :],
                                            in1=tu[:, :], op=ALU.subtract)
                    P = hp.tile([64, NB * 256], F32, tag=f"p{s}")
                    nc.vector.scalar_tensor_tensor(
                        out=P[:, :], in0=tv[:, :], scalar=PI, in1=ta[:, :],
                        op0=ALU.mult, op1=ALU.add)
                    Ps.append(P)

                # cosine-sim stats: per-block sums of p1*p2, p1^2, p2^2
                R3 = tp.tile([64, 3, NB], F32, tag="r3")
                pm = tp.tile([64, NB * 256], F32, tag="phD")
                for si, (ia, ib) in enumerate(((0, 1), (0, 0), (1, 1))):
                    nc.vector.tensor_tensor(out=pm[:, :], in0=Ps[ia][:, :],
                                            in1=Ps[ib][:, :], op=ALU.mult)
                    nc.vector.tensor_reduce(
                        R3[:, si, :], pm[:, :].rearrange("p (a b) -> p a b", a=NB),
                        axis=AXL.X, op=ALU.add)
                pS = pap.tile([1, 3 * NB], F32, tag="pa")
                nc.tensor.matmul(pS[:, :], ones64t[:, :],
                                 R3[:, :, :].rearrange("p a b -> p (a b)"),
                                 start=True, stop=True)
                nc.vector.tensor_copy(STATS[:, j * 24:(j + 1) * 24], pS[:, :])

                # L1 partial sums for this iteration's 8 blocks
                pqc = xp.tile([96, 256], BF16, tag="pqc")
                nc.sync.dma_start(pqc[:, :], pq_d[:, j * 256:(j + 1) * 256])
                tqc = xp.tile([96, 256], BF16, tag="tqc")
                nc.sync.dma_start(tqc[:, :], tq_d[:, j * 256:(j + 1) * 256])
                dl = tp.tile([96, 256], F32, tag="dl")
                nc.vector.tensor_tensor(out=dl[:, :], in0=pqc[:, :],
                                        in1=tqc[:, :], op=ALU.subtract)
                nc.vector.tensor_reduce(
                    L1S[:, j * 8:(j + 1) * 8],
                    dl[:, :].rearrange("p (a b) -> p a b", a=8),
                    axis=AXL.X, op=ALU.add, apply_absolute_value=True)

            # ---- epilogue (all tiny, partition 0) ----
            pL = pap.tile([1, 128], F32, tag="pa")
            nc.tensor.matmul(pL[:, :], ones96t[:, :], L1S[:, :], start=True, stop=True)
            R1 = tp.tile([1, 128], F32, tag="r1")
            nc.vector.tensor_copy(R1[:, :], pL[:, :])
            SV = STATS[:, :].rearrange("p (j s t) -> p j s t", s=3, t=8)
            NUM = SV[:, :, 0, :]
            N1 = SV[:, :, 1, :]
            N2 = SV[:, :, 2, :]
            W_ = tp.tile([1, 16, 8], F32, tag="w_")
            nc.vector.tensor_tensor(out=W_[:, :, :], in0=N1, in1=N2, op=ALU.mult)
            S2 = tp.tile([1, 16, 8], F32, tag="s2")
            nc.vector.tensor_tensor(out=S2[:, :, :], in0=NUM, in1=NUM, op=ALU.mult)
            C2 = tp.tile([1, 16, 8], F32, tag="c2")
            nc.vector.scalar_tensor_tensor(
                out=C2[:, :, :], in0=S2[:, :, :], scalar=25.0, in1=W_[:, :, :],
                op0=ALU.mult, op1=ALU.is_ge)
            M1 = tp.tile([1, 16, 8], F32, tag="m1")
            nc.vector.tensor_scalar(out=M1[:, :, :], in0=NUM, scalar1=0.0,
                                    scalar2=None, op0=ALU.is_gt)
            MASK = tp.tile([1, 128], F32, tag="mask")
            nc.vector.tensor_tensor(
                out=MASK[:, :].rearrange("p (a b) -> p a b", a=16),
                in0=C2[:, :, :], in1=M1[:, :, :], op=ALU.mult)
            PR_ = tp.tile([1, 128], F32, tag="pr_")
            nc.vector.tensor_tensor(out=PR_[:, :], in0=MASK[:, :],
                                    in1=R1[:, :], op=ALU.mult)
            OT = tp.tile([1, 2], F32, tag="ot")
            nc.vector.tensor_reduce(OT[:, 0:1], PR_[:, :], axis=AXL.X, op=ALU.add)
            nc.vector.tensor_reduce(OT[:, 1:2], MASK[:, :], axis=AXL.X, op=ALU.add)
            nc.sync.dma_start(o_d[:, :], OT[:, :])
    nc.compile()
    return nc


def _blocks(x, c):
    # [16, c, 256, 256] -> [1024, c, 32, 32], block = img*64 + by*8 + bx
    return (x.reshape(16, c, 8, 32, 8, 32).transpose(0, 2, 4, 1, 3, 5)
            .reshape(1024, c, 32, 32))


def kernel(pred1, pred2, target, w1, b1, w2, b2, w3, b3, w4, b4, w5, b5):
    import ml_dtypes
    from concourse.bass_utils import run_bass_kernel_spmd
    bf = ml_dtypes.bfloat16

    pred1 = np.asarray(pred1, np.float32)
    pred2 = np.asarray(pred2, np.float32)
    target = np.asarray(target, np.float32)
    w1, w2, w3, w4, w5 = (np.asarray(w, np.float32) for w in (w1, w2, w3, w4, w5))
    b1, b2, b3, b4, b5 = (np.asarray(b, np.float32) for b in (b1, b2, b3, b4, b5))

    def taps_T(w):  # [O, I, 3, 3] -> [9, I, O]
        return np.ascontiguousarray(
            w.transpose(2, 3, 1, 0).reshape(9, w.shape[1], w.shape[0]))

    w1T, w2T, w3T, w4T, w5T = map(taps_T, (w1, w2, w3, w4, w5))
    w27 = np.ascontiguousarray(w1T.reshape(27, 64)).astype(bf)
    w12p = np.stack([np.concatenate([w2T[3 * dy], w2T[3 * dy + 1]], 0)
                     for dy in range(3)]).astype(bf)
    w12s = np.stack([w2T[3 * dy + 2] for dy in range(3)]).astype(bf)
    w21p = np.stack([np.concatenate([w3T[3 * dy], w3T[3 * dy + 1]], 0)
                     for dy in range(3)]).astype(bf)
    w21s = np.stack([w3T[3 * dy + 2] for dy in range(3)]).astype(bf)
    w22 = w4T.astype(bf)
    w5p = w5T.astype(bf)

    idx = np.arange(8)
    s_in = (idx[:, None, None, None] * idx[None, None, :, None]
            + idx[None, :, None, None] * idx[None, None, None, :])  # [ri,ci,ro,co]
    theta = 2.0 * np.pi * (s_in % 8) / 8.0
    M_r = np.cos(theta).reshape(64, 64).astype(np.float32)
    M_i = (-np.sin(theta)).reshape(64, 64).astype(np.float32)
    M_r[np.abs(M_r) < 1e-6] = 0.0
    M_i[np.abs(M_i) < 1e-6] = 0.0
    # 10-pitch layout: row p = r*10 + c holds spatial (r,c); pitch cols 8,9 = 0
    dftr = np.zeros((80, 64), np.float32)
    dfti = np.zeros((80, 64), np.float32)
    for r in range(8):
        dftr[r * 10:r * 10 + 8, :] = M_r[r * 8:(r + 1) * 8, :]
        dfti[r * 10:r * 10 + 8, :] = M_i[r * 8:(r + 1) * 8, :]
    b5bc = np.tile(b5.reshape(1, 256), (80, 1)).astype(np.float32)

    shared = dict(
        w27=w27, w12p=w12p, w12s=w12s, w21p=w21p, w21s=w21s, w22=w22, w5=w5p,
        dftr=dftr.astype(bf), dfti=dfti.astype(bf),
        b1=np.ascontiguousarray(b1.reshape(64, 1)),
        b2=np.ascontiguousarray(b2.reshape(64, 1)),
        b3=np.ascontiguousarray(b3.reshape(128, 1)),
        b4=np.ascontiguousarray(b4.reshape(128, 1)), b5bc=b5bc)

    p1n = ((pred1 - MEAN) / STD).astype(np.float32)
    tgn = ((target - MEAN) / STD).astype(np.float32)
    bl1 = _blocks(p1n, 3)
    blt = _blocks(tgn, 3)
    blp2 = _blocks(pred2, 3)
    bltg = _blocks(target, 3)

    def pad36(blk):  # [128, 3, 32, 32] -> [3, 128, 1296] bf16
        out = np.zeros((3, 128, 36, 36), np.float32)
        out[:, :, 2:34, 2:34] = blk.transpose(1, 0, 2, 3)
        return out.reshape(3, 128, 1296).astype(bf)

    def l1fmt(blk):  # [128, 3, 32, 32] -> [96, 4096] bf16
        return np.ascontiguousarray(
            blk.transpose(1, 2, 0, 3).reshape(96, 128 * 32)).astype(bf)

    in_maps = []
    for c in range(N_CORES):
        s = slice(c * 128, (c + 1) * 128)
        in_maps.append(dict(
            xa=pad36(bl1[s]), xb=pad36(blt[s]),
            pq=l1fmt(blp2[s]), tq=l1fmt(bltg[s]), **shared))

    if "nc" not in _COMPILED:
        _COMPILED["nc"] = _build_nc()
    nc = _COMPILED["nc"]

    import time as _time
    t0 = _time.perf_counter()
    try:
        res = run_bass_kernel_spmd(nc, in_maps, list(range(N_CORES)), trace=PROFILE)
    except ModuleNotFoundError:
        res = run_bass_kernel_spmd(nc, in_maps, list(range(N_CORES)))
    global LAST_EXEC_NS, LAST_TRACE_PATH
    LAST_EXEC_NS = int((_time.perf_counter() - t0) * 1e9)
    if res.exec_time_ns:
        LAST_EXEC_NS = int(res.exec_time_ns)
    if res.instructions_and_trace:
        LAST_TRACE_PATH = res.instructions_and_trace[1]

    l1 = 0.0
    cnt = 0.0
    for c in range(N_CORES):
        o = res.results[c]["o"]
        l1 += float(o[0, 0])
        cnt += float(o[0, 1])
    out = np.float32(l1 / (cnt * 1024.0 + 1e-6))
    return np.array(out, dtype=np.float32)
